# revision 30
# baseline (speedup 1.0000x reference)
"""GQA sigmoid-attention (causal zero-fill) Trainium2 Bass kernel.

Problem: nn_Attention (B=2, S=2048, D=2048, 16 q-heads / 4 kv-heads, head_dim=128)
    xq = query @ Wq.T ; xk = key @ Wk.T ; xv = value @ Wv.T   (GQA repeat 4x)
    scores = sigmoid((xq xk^T) / sqrt(128)); causal zero-fill AFTER sigmoid
    out = (scores @ xv) @ Wo.T

Sharding (8 NeuronCores): core = (b, g) with b in {0,1} batches and g in {0..3}
kv-groups. Each core owns 4 query heads + their 1 kv head for one batch and
computes a partial output [S, D] through its Wo row-slice; the host sums the 4
partials per batch (the "all-reduce" of the row-sharded Wo).

All matmul operands are bf16 (PE: 1 cycle/row at any width; psum accumulation
stays f32). The host pre-casts and pre-packs every input to the exact SBUF
layout (free, outside device timing), so the device does no dtype conversion
on loads and half the HBM traffic of fp32.

DMA queue split (the key to keeping PE fed):
  SP/HWDGE    : qc/kc/vc input streams, dt-quad chunks, deep ring buffers
  ACT/HWDGE   : the 4 weight loads, issued at t=0 (ACT idles until B(0))
  Pool/SWDGE  : output writes (their data-ready waits head-of-line block the
                issuing queue, so they get a queue nothing else needs)

Per q-tile j (512 rows): projections (dt-quad streamed; Qproj leads at j=0
since only wq chunks + q quads gate the first PE work, KVproj leads for
j>=1 so stream buffers recycle at tile start) -> PE-transpose of V ->
B(j), kc-outer: per k-chunk the 4 heads' scoresT[k,q] matmuls share the
xkT[kc] stationary and their attnT accumulations share xv[kc] (fewer
distinct PE weight loads), sigmoid (ACT, scale folded, psum->bf16) ->
diagonal mask (DVE mul); four sigmoids in flight decouple ACT latency from
PE. C(j-1) output-projection groups are spread across the kc loop (they use
the kv PSUM pair; ps8 holds the 4 attn accumulators). Diagonal score tiles
are trimmed to the unmasked columns (c0 = 128*r, bf16 has no narrow-tile
penalty); fully-masked tiles are skipped (halves attention FLOPs).
"""

import math

import ml_dtypes
import numpy as np

import concourse.bacc as bacc
import concourse.mybir as mybir
import concourse.tile as tile
from concourse.bass_utils import run_bass_kernel_spmd
from concourse.masks import make_identity

B = 2
S = 2048
D = 2048
NH = 16
NKV = 4
C = 128          # head dim
HPG = NH // NKV  # 4 query heads per kv group (= per core)
F = HPG * C      # 512 query-proj dims per core
SCALE = 1.0 / math.sqrt(C)
P = 128
DT = D // P      # 16 contraction chunks
J4 = S // 512    # 4 query tiles of 512
ST = S // P      # 16 s-chunks
NQ = 4           # dt chunks per stream DMA (quad)

f32 = mybir.dt.float32
bf16 = mybir.dt.bfloat16

_CACHE: dict = {}

_OPTS = {"phases": "ABC", "c_interleave": True, "q_bufs": 8, "kv_bufs": 10,
         "oe_bufs": 3, "pr_bufs": 8}


def _build_module(n_iters: int = 0, internal_io: bool = False):
    """Build the per-core module. n_iters=0: straight-line kernel (production).
    n_iters>0: wrap the whole body in a For_i repeat loop (timing variant —
    per-iteration wall-clock slope measures true on-device exec time).
    internal_io=True replaces the big I/O tensors with on-device scratch so
    a timing call transfers almost nothing over the wire."""
    nc = bacc.Bacc("TRN2", target_bir_lowering=False, debug=False, num_devices=8)

    if internal_io:
        dummy_in = nc.dram_tensor("dummy_in", [1, 1], f32, kind="ExternalInput")
        dummy_out = nc.dram_tensor("dummy_out", [1, 1], f32, kind="ExternalOutput")
        kw = {}
    else:
        kw = {"kind": "ExternalInput"}
    qT = nc.dram_tensor("qT", [D, S], bf16, **kw)
    kT = nc.dram_tensor("kT", [D, S], bf16, **kw)
    vT = nc.dram_tensor("vT", [D, S], bf16, **kw)
    wqP = nc.dram_tensor("wqP", [P, DT, F], bf16, **kw)
    wkP = nc.dram_tensor("wkP", [P, DT, C], bf16, **kw)
    wvP = nc.dram_tensor("wvP", [P, DT, C], bf16, **kw)
    woP = nc.dram_tensor("woP", [P, HPG, D], bf16, **kw)
    if internal_io:
        out = nc.dram_tensor("out", [S, D], bf16)
    else:
        out = nc.dram_tensor("out", [S, D], bf16, kind="ExternalOutput")

    qT_r = qT.rearrange("(dt p) s -> p dt s", p=P)
    kT_r = kT.rearrange("(dt p) s -> p dt s", p=P)
    vT_r = vT.rearrange("(dt p) s -> p dt s", p=P)

    do_B = "B" in _OPTS["phases"]
    do_C = "C" in _OPTS["phases"]

    with tile.TileContext(nc) as tc:
        with (
            tc.tile_pool(name="consts", bufs=1) as consts,
            tc.tile_pool(name="weights", bufs=1) as wpool,
            tc.tile_pool(name="xkv", bufs=1) as xkv_pool,
            tc.tile_pool(name="xq", bufs=2) as xq_pool,
            tc.tile_pool(name="attn_sb", bufs=2) as apool,
            tc.tile_pool(name="qstream", bufs=_OPTS["q_bufs"]) as qstream,
            tc.tile_pool(name="kvstream", bufs=_OPTS["kv_bufs"]) as kvstream,
            tc.tile_pool(name="vtr", bufs=2) as vtr,
            tc.tile_pool(name="probs", bufs=_OPTS["pr_bufs"]) as probs,
            tc.tile_pool(name="oevac", bufs=_OPTS["oe_bufs"]) as oevac,
            tc.tile_pool(name="ps8", bufs=4, space="PSUM") as ps8,
            tc.tile_pool(name="ps_kv", bufs=2, space="PSUM") as ps_kv_pool,
            tc.tile_pool(name="ps_sc", bufs=2, space="PSUM") as ps_sc_pool,
        ):
          def emit_c(at_prev, j, s16, last=False, kv_only=False):
              """C(j, s16): one 128-row group of the output projection.
              n4 pairs share the stationary operand per h (LDW amortized).
              Output leaves as bf16, two 512-col halves batched per DMA;
              the final tile's writes go on the then-idle SP queue.
              kv_only: keep both PSUM groups on the kv pool (the kc-outer
              B loop owns all four ps8 banks)."""
              row0 = (j * 4 + s16) * P
              for np_ in range(2):
                  pool_ = ps_kv_pool if kv_only or np_ else ps8
                  tag_ = "kv" if kv_only or np_ else "x"
                  ps_o = [pool_.tile([P, 512], f32, tag=tag_, name=f"ps_o{i}")
                          for i in range(2)]
                  for h in range(HPG):
                      for i in range(2):
                          n4 = np_ * 2 + i
                          nc.tensor.matmul(
                              ps_o[i][:],
                              at_prev[:, h, s16 * P:(s16 + 1) * P],
                              wo_sb[:, h, n4 * 512:(n4 + 1) * 512],
                              start=(h == 0), stop=(h == HPG - 1))
                  ot = oevac.tile([P, 1024], bf16, tag="ot", name="ot")
                  for i in range(2):
                      nc.vector.tensor_copy(ot[:, i * 512:(i + 1) * 512],
                                            ps_o[i][:])
                  # final tile: alternate between the two then-idle queues
                  eng = (nc.sync if np_ == 0 else nc.gpsimd) if last else nc.gpsimd
                  eng.dma_start(
                      out[row0:row0 + P, np_ * 1024:(np_ + 1) * 1024], ot[:])

          def body(_iv=None):
            global wo_sb
            # weight loads on the ACT HWDGE queue at t=0: ACT is idle until
            # B(0), and big weight transfers must not block the input stream.
            # wq arrives in dt-quad chunks so Qproj(0) starts ~3us in; wo is
            # last (first needed at C(0), well after startup).
            wk_sb = wpool.tile([P, DT, C], bf16, tag="wk", name="wk_sb")
            wv_sb = wpool.tile([P, DT, C], bf16, tag="wv", name="wv_sb")
            wq_sb = wpool.tile([P, DT, F], bf16, tag="wq", name="wq_sb")
            wo_sb = wpool.tile([P, HPG, D], bf16, tag="wo", name="wo_sb")
            # first quad as two pairs so Qproj(0)'s dt=0 matmul starts sooner
            for d0, d1 in [(0, 2), (2, 4)] + [(NQ * qd, NQ * (qd + 1))
                                              for qd in range(1, DT // NQ)]:
                nc.scalar.dma_start(wq_sb[:, d0:d1, :], wqP[:, d0:d1, :])
            nc.scalar.dma_start(wk_sb[:], wkP[:])
            nc.scalar.dma_start(wv_sb[:], wvP[:])
            nc.scalar.dma_start(wo_sb[:], woP[:])

            ident = consts.tile([P, P], bf16, name="ident")
            masks = consts.tile([P, J4, 512], bf16, name="masks")
            make_identity(nc, ident)
            # causal masks for the diagonal 128x512 tiles: keep (k <= q)
            # i.e. mask_r[i, jq] = 1 iff jq - i - 128 r >= 0
            nc.gpsimd.memset(masks[:], 1.0)
            for r in range(J4):
                nc.gpsimd.affine_select(
                    out=masks[:, r, :], in_=masks[:, r, :],
                    compare_op=mybir.AluOpType.is_ge,
                    fill=0.0, base=-P * r, channel_multiplier=-1,
                    pattern=[[1, 512]])

            xkT = xkv_pool.tile([P, S], bf16, tag="xkT", name="xkT")    # [c,k]
            xv = xkv_pool.tile([P, ST, C], bf16, tag="xv", name="xv")   # [k%128,kc,c]

            at_prev = None
            for j in range(J4):
                sl_ = slice(j * 512, (j + 1) * 512)
                xqT_j = xq_pool.tile([P, HPG, 512], bf16, tag="xqT", name="xqT_j")

                def qproj():
                    # at startup only wq chunks + q quads must land before
                    # PE can work, so Qproj leads at j=0
                    ps_q = [ps8.tile([P, 512], f32, tag="x", name=f"psq{h_}")
                            for h_ in range(HPG)]
                    chunks = ([(0, 2), (2, 4)] if j == 0 else [(0, NQ)]) + \
                        [(NQ * qd, NQ * (qd + 1)) for qd in range(1, DT // NQ)]
                    for d0, d1 in chunks:
                        qc = qstream.tile([P, NQ, 512], bf16, tag="qc", name="qc")
                        nc.sync.dma_start(qc[:, :d1 - d0, :], qT_r[:, d0:d1, sl_])
                        for i in range(d1 - d0):
                            dt = d0 + i
                            for h in range(HPG):
                                nc.tensor.matmul(
                                    ps_q[h][:], wq_sb[:, dt, h * P:(h + 1) * P],
                                    qc[:, i, :], start=(dt == 0),
                                    stop=(dt == DT - 1))
                    for h in range(HPG):
                        nc.vector.tensor_copy(xqT_j[:, h, :], ps_q[h][:])

                def kvproj():
                    # KV leads for j>=1: stream buffers recycle at tile
                    # start, so the next tile's transfers spread into the
                    # B-phase DMA lull instead of bunching just-in-time
                    ps_k = ps_kv_pool.tile([P, 512], f32, tag="kv", name="ps_k")
                    ps_v = ps_kv_pool.tile([P, 512], f32, tag="kv", name="ps_v")
                    for qd in range(DT // NQ):
                        kc = kvstream.tile([P, NQ, 512], bf16, tag="kc", name="kc")
                        vc = kvstream.tile([P, NQ, 512], bf16, tag="vc", name="vc")
                        nc.sync.dma_start(kc[:], kT_r[:, NQ * qd:NQ * (qd + 1), sl_])
                        nc.sync.dma_start(vc[:], vT_r[:, NQ * qd:NQ * (qd + 1), sl_])
                        for i in range(NQ):
                            dt = NQ * qd + i
                            st, sp = dt == 0, dt == DT - 1
                            nc.tensor.matmul(ps_k[:], wk_sb[:, dt, :], kc[:, i, :],
                                             start=st, stop=sp)
                            nc.tensor.matmul(ps_v[:], wv_sb[:, dt, :], vc[:, i, :],
                                             start=st, stop=sp)
                    nc.vector.tensor_copy(xkT[:, sl_], ps_k[:])
                    xvT_sb = vtr.tile([P, 512], bf16, tag="xvT", name="xvT_sb")
                    nc.vector.tensor_copy(xvT_sb[:], ps_v[:])
                    for sc in range(4):
                        pst = ps_kv_pool.tile([P, P], bf16, tag="kv", name="pst")
                        nc.tensor.transpose(pst[:], xvT_sb[:, sc * P:(sc + 1) * P],
                                            ident[:])
                        nc.vector.tensor_copy(xv[:, j * 4 + sc, :], pst[:])

                if j == 0:
                    qproj(); kvproj()
                else:
                    kvproj(); qproj()

                if not do_B:
                    continue
                nk = 4 * (j + 1)
                at_block = apool.tile([P, HPG, 512], bf16, tag="attnT",
                                      name="at_block")

                def score_prob(kc_i, h, sc_pool=None, sc_tag="sc"):
                    # diagonal tiles (r >= 0): columns < 128 r are fully
                    # masked -> compute only cols >= 128 r
                    r = kc_i - 4 * j
                    c0 = 128 * r if r > 0 else 0
                    pool_ = sc_pool or ps_sc_pool
                    ps_s = pool_.tile([P, 512], f32, tag=sc_tag, name="ps_s")
                    nc.tensor.matmul(
                        ps_s[:, c0:], xkT[:, kc_i * P:(kc_i + 1) * P],
                        xqT_j[:, h, c0:], start=True, stop=True)
                    pr = probs.tile([P, 512], bf16, tag="pr", name="pr")
                    nc.scalar.activation(
                        pr[:, c0:], ps_s[:, c0:],
                        mybir.ActivationFunctionType.Sigmoid,
                        scale=float(SCALE))
                    if r >= 0:
                        nc.vector.tensor_mul(
                            out=pr[:, c0:], in0=pr[:, c0:], in1=masks[:, r, c0:])
                    return pr, c0

                # kc-outer: the 4 heads' score matmuls share the xkT[kc]
                # stationary and their AVs share xv[kc] (fewer distinct
                # weight loads); 4 sigmoids of slack decouple ACT latency.
                # C(j-1) groups are spread across the kc loop; they use only
                # the kv PSUM pair since ps8 holds the 4 attn accumulators.
                if _OPTS.get("b_order", "kc") == "kc" or at_prev is None:
                    ps_at = [ps8.tile([P, 512], f32, tag="x", name=f"ps_at{h_}")
                             for h_ in range(HPG)]
                    cpoints = {}
                    if do_C and _OPTS["c_interleave"] and at_prev is not None:
                        cpoints = {(nk * (i + 1)) // 4 - 1: i for i in range(4)}
                    for kc_i in range(nk):
                        if at_prev is None:
                            # B(0): the kv PSUM pair is idle, double the
                            # score pipeline with it
                            prs = [score_prob(kc_i, h,
                                              sc_pool=(ps_kv_pool if h % 2 else None),
                                              sc_tag=("kv" if h % 2 else "sc"))
                                   for h in range(HPG)]
                        else:
                            prs = [score_prob(kc_i, h) for h in range(HPG)]
                        for h in range(HPG):
                            pr, c0 = prs[h]
                            nc.tensor.matmul(ps_at[h][:, c0:], xv[:, kc_i, :],
                                             pr[:, c0:], start=(kc_i == 0),
                                             stop=(kc_i == nk - 1))
                        if kc_i in cpoints:
                            emit_c(at_prev, j - 1, cpoints[kc_i], kv_only=True)
                    for h in range(HPG):
                        nc.vector.tensor_copy(at_block[:, h, :], ps_at[h][:])
                else:
                    # h-outer: one attn accumulator, C(j-1) per-head
                    for h in range(HPG):
                        ps_at = ps8.tile([P, 512], f32, tag="x", name="ps_at")
                        for kc_i in range(nk):
                            pr, c0 = score_prob(kc_i, h)
                            nc.tensor.matmul(ps_at[:, c0:], xv[:, kc_i, :],
                                             pr[:, c0:], start=(kc_i == 0),
                                             stop=(kc_i == nk - 1))
                        nc.vector.tensor_copy(at_block[:, h, :], ps_at[:])
                        if do_C and _OPTS["c_interleave"] and at_prev is not None:
                            emit_c(at_prev, j - 1, h)
                if do_C and not _OPTS["c_interleave"]:
                    for s16 in range(4):
                        emit_c(at_block, j, s16)
                at_prev = at_block

            if do_B and do_C and _OPTS["c_interleave"]:
                # kv_only: ps8 frees right after B(3)'s evacs, so the next
                # For_i iteration's Qproj matmuls can flow into this C tail
                for s16 in range(4):
                    emit_c(at_prev, J4 - 1, s16, last=True, kv_only=True)

          if internal_io:
              dt_ = consts.tile([1, 1], f32, name="dt_")
              nc.sync.dma_start(dt_[:], dummy_in[:])
              nc.sync.dma_start(dummy_out[:], dt_[:])
          if n_iters:
              import os as _os
              _kw = {}
              if _os.environ.get("LOOP_HINTS") == "1":
                  _kw = dict(hint_engines=(mybir.EngineType.PE,
                                           mybir.EngineType.Activation,
                                           mybir.EngineType.DVE,
                                           mybir.EngineType.Pool,
                                           mybir.EngineType.SP))
              if _os.environ.get("LOOP_STAGGER") == "1":
                  _kw["staggered_reset"] = True
              with tc.For_i(0, n_iters, 1, **_kw):
                  body()
          else:
              body()
    nc.compile()
    return nc


def _get_module():
    if "nc" not in _CACHE:
        _CACHE["nc"] = _build_module()
    return _CACHE["nc"]


def _bf16(a: np.ndarray) -> np.ndarray:
    return np.ascontiguousarray(a.astype(ml_dtypes.bfloat16))


def _pack_w(wT: np.ndarray, free: int) -> np.ndarray:
    """[D, free] weight (already W.T slice) -> SBUF-layout [128, DT, free]."""
    return _bf16(wT.reshape(DT, P, free).transpose(1, 0, 2))


def make_in_maps(query, key, value, Wq, Wk, Wv, Wo):
    """Host-side sharding: per-core input dict (core = b*4 + g)."""
    query = np.asarray(query, dtype=np.float32)
    key = np.asarray(key, dtype=np.float32)
    value = np.asarray(value, dtype=np.float32)
    Wq = np.asarray(Wq, dtype=np.float32)
    Wk = np.asarray(Wk, dtype=np.float32)
    Wv = np.asarray(Wv, dtype=np.float32)
    Wo = np.asarray(Wo, dtype=np.float32)

    qT = [_bf16(query[b].T) for b in range(B)]
    kTb = [_bf16(key[b].T) for b in range(B)]
    vTb = [_bf16(value[b].T) for b in range(B)]
    WqT = Wq.T  # [D, NH*C]
    WkT = Wk.T  # [D, NKV*C]
    WvT = Wv.T
    WoT = Wo.T  # [D_in, D_out]

    in_maps = []
    for core in range(8):
        b, g = divmod(core, 4)
        woT_g = WoT[g * F:(g + 1) * F, :]  # [F, D]
        in_maps.append({
            "qT": qT[b],
            "kT": kTb[b],
            "vT": vTb[b],
            "wqP": _pack_w(WqT[:, g * F:(g + 1) * F], F),
            "wkP": _pack_w(WkT[:, g * C:(g + 1) * C], C),
            "wvP": _pack_w(WvT[:, g * C:(g + 1) * C], C),
            # [F, D] -> [128, HPG, D] (partition = c within head chunk)
            "woP": _bf16(woT_g.reshape(HPG, P, D).transpose(1, 0, 2)),
        })
    return in_maps


def kernel(**inputs) -> np.ndarray:
    nc = _get_module()
    in_maps = make_in_maps(**inputs)
    res = run_bass_kernel_spmd(nc, in_maps, core_ids=list(range(8)))
    parts = [np.asarray(res.results[c]["out"], dtype=np.float32)
             for c in range(8)]
    full = np.empty((B, S, D), dtype=np.float32)
    for b in range(B):
        full[b] = parts[b * 4] + parts[b * 4 + 1] + parts[b * 4 + 2] + parts[b * 4 + 3]
    return full


# revision 31
# speedup vs baseline: 1.3170x; 1.3170x over previous
"""GQA sigmoid-attention (causal zero-fill) Trainium2 Bass kernel.

Problem: nn_Attention (B=2, S=2048, D=2048, 16 q-heads / 4 kv-heads, head_dim=128)
    xq = query @ Wq.T ; xk = key @ Wk.T ; xv = value @ Wv.T   (GQA repeat 4x)
    scores = sigmoid((xq xk^T) / sqrt(128)); causal zero-fill AFTER sigmoid
    out = (scores @ xv) @ Wo.T

Sharding (8 NeuronCores): core = (b, g) with b in {0,1} batches and g in {0..3}
kv-groups. Each core owns 4 query heads + their 1 kv head for one batch and
computes a partial output [S, D] through its Wo row-slice; the host sums the 4
partials per batch (the "all-reduce" of the row-sharded Wo).

All matmul operands are bf16 (PE: 1 cycle/row at any width; psum accumulation
stays f32). The host pre-casts and pre-packs every input to the exact SBUF
layout (free, outside device timing), so the device does no dtype conversion
on loads and half the HBM traffic of fp32.

DMA queue split (the key to keeping PE fed):
  SP/HWDGE    : qc/kc/vc input streams, dt-quad chunks, deep ring buffers
  ACT/HWDGE   : the 4 weight loads, issued at t=0 (ACT idles until B(0))
  Pool/SWDGE  : output writes (their data-ready waits head-of-line block the
                issuing queue, so they get a queue nothing else needs)

Per q-tile j (512 rows): projections (dt-quad streamed; Qproj leads at j=0
since only wq chunks + q quads gate the first PE work, KVproj leads for
j>=1 so stream buffers recycle at tile start) -> PE-transpose of V ->
B(j), kc-outer: per k-chunk the 4 heads' scoresT[k,q] matmuls share the
xkT[kc] stationary and their attnT accumulations share xv[kc] (fewer
distinct PE weight loads), sigmoid (ACT, scale folded, psum->bf16) ->
diagonal mask (DVE mul); four sigmoids in flight decouple ACT latency from
PE. C(j-1) output-projection groups are spread across the kc loop (they use
the kv PSUM pair; ps8 holds the 4 attn accumulators). Diagonal score tiles
are trimmed to the unmasked columns (c0 = 128*r, bf16 has no narrow-tile
penalty); fully-masked tiles are skipped (halves attention FLOPs).
"""

import math

import ml_dtypes
import numpy as np

import concourse.bacc as bacc
import concourse.mybir as mybir
import concourse.tile as tile
from concourse.bass_utils import run_bass_kernel_spmd
from concourse.masks import make_identity

B = 2
S = 2048
D = 2048
NH = 16
NKV = 4
C = 128          # head dim
HPG = NH // NKV  # 4 query heads per kv group (= per core)
F = HPG * C      # 512 query-proj dims per core
SCALE = 1.0 / math.sqrt(C)
P = 128
DT = D // P      # 16 contraction chunks
J4 = S // 512    # 4 query tiles of 512
ST = S // P      # 16 s-chunks
NQ = 4           # dt chunks per stream DMA (quad)

f32 = mybir.dt.float32
bf16 = mybir.dt.bfloat16

_CACHE: dict = {}

_OPTS = {"phases": "ABC", "c_interleave": True, "q_bufs": 8, "kv_bufs": 10,
         "oe_bufs": 3, "pr_bufs": 8}


def _build_module(n_iters: int = 0, internal_io: bool = False):
    """Build the per-core module. n_iters=0: straight-line kernel (production).
    n_iters>0: wrap the whole body in a For_i repeat loop (timing variant —
    per-iteration wall-clock slope measures true on-device exec time).
    internal_io=True replaces the big I/O tensors with on-device scratch so
    a timing call transfers almost nothing over the wire."""
    nc = bacc.Bacc("TRN2", target_bir_lowering=False, debug=False, num_devices=8)

    if internal_io:
        dummy_in = nc.dram_tensor("dummy_in", [1, 1], f32, kind="ExternalInput")
        dummy_out = nc.dram_tensor("dummy_out", [1, 1], f32, kind="ExternalOutput")
        kw = {}
    else:
        kw = {"kind": "ExternalInput"}
    qT = nc.dram_tensor("qT", [D, S], bf16, **kw)
    kT = nc.dram_tensor("kT", [D, S], bf16, **kw)
    vT = nc.dram_tensor("vT", [D, S], bf16, **kw)
    wqP = nc.dram_tensor("wqP", [P, DT, F], bf16, **kw)
    wkP = nc.dram_tensor("wkP", [P, DT, C], bf16, **kw)
    wvP = nc.dram_tensor("wvP", [P, DT, C], bf16, **kw)
    woP = nc.dram_tensor("woP", [P, HPG, D], bf16, **kw)
    if internal_io:
        out = nc.dram_tensor("out", [S, D], bf16)
    else:
        out = nc.dram_tensor("out", [S, D], bf16, kind="ExternalOutput")

    qT_r = qT.rearrange("(dt p) s -> p dt s", p=P)
    kT_r = kT.rearrange("(dt p) s -> p dt s", p=P)
    vT_r = vT.rearrange("(dt p) s -> p dt s", p=P)

    do_B = "B" in _OPTS["phases"]
    do_C = "C" in _OPTS["phases"]

    with tile.TileContext(nc) as tc:
        with (
            tc.tile_pool(name="consts", bufs=1) as consts,
            tc.tile_pool(name="weights", bufs=1) as wpool,
            tc.tile_pool(name="xkv", bufs=1) as xkv_pool,
            tc.tile_pool(name="xq", bufs=2) as xq_pool,
            tc.tile_pool(name="attn_sb", bufs=2) as apool,
            tc.tile_pool(name="qstream", bufs=_OPTS["q_bufs"]) as qstream,
            tc.tile_pool(name="kvstream", bufs=_OPTS["kv_bufs"]) as kvstream,
            tc.tile_pool(name="vtr", bufs=2) as vtr,
            tc.tile_pool(name="probs", bufs=_OPTS["pr_bufs"]) as probs,
            tc.tile_pool(name="oevac", bufs=_OPTS["oe_bufs"]) as oevac,
            tc.tile_pool(name="ps8", bufs=4, space="PSUM") as ps8,
            tc.tile_pool(name="ps_kv", bufs=2, space="PSUM") as ps_kv_pool,
            tc.tile_pool(name="ps_sc", bufs=2, space="PSUM") as ps_sc_pool,
        ):
          def emit_c(at_prev, j, s16, last=False, kv_only=False):
              """C(j, s16): one 128-row group of the output projection.
              n4 pairs share the stationary operand per h (LDW amortized).
              Output leaves as bf16, two 512-col halves batched per DMA;
              the final tile's writes go on the then-idle SP queue.
              kv_only: keep both PSUM groups on the kv pool (the kc-outer
              B loop owns all four ps8 banks)."""
              row0 = (j * 4 + s16) * P
              for np_ in range(2):
                  pool_ = ps_kv_pool if kv_only or np_ else ps8
                  tag_ = "kv" if kv_only or np_ else "x"
                  ps_o = [pool_.tile([P, 512], f32, tag=tag_, name=f"ps_o{i}")
                          for i in range(2)]
                  for h in range(HPG):
                      for i in range(2):
                          n4 = np_ * 2 + i
                          nc.tensor.matmul(
                              ps_o[i][:],
                              at_prev[:, h, s16 * P:(s16 + 1) * P],
                              wo_sb[:, h, n4 * 512:(n4 + 1) * 512],
                              start=(h == 0), stop=(h == HPG - 1))
                  ot = oevac.tile([P, 1024], bf16, tag="ot", name="ot")
                  for i in range(2):
                      nc.vector.tensor_copy(ot[:, i * 512:(i + 1) * 512],
                                            ps_o[i][:])
                  # final tile: alternate between the two then-idle queues
                  eng = (nc.sync if np_ == 0 else nc.gpsimd) if last else nc.gpsimd
                  eng.dma_start(
                      out[row0:row0 + P, np_ * 1024:(np_ + 1) * 1024], ot[:])

          def body(_iv=None):
            global wo_sb
            # weight loads on the ACT HWDGE queue at t=0: ACT is idle until
            # B(0), and big weight transfers must not block the input stream.
            # wq arrives in dt-quad chunks so Qproj(0) starts ~3us in; wo is
            # last (first needed at C(0), well after startup).
            wk_sb = wpool.tile([P, DT, C], bf16, tag="wk", name="wk_sb")
            wv_sb = wpool.tile([P, DT, C], bf16, tag="wv", name="wv_sb")
            wq_sb = wpool.tile([P, DT, F], bf16, tag="wq", name="wq_sb")
            wo_sb = wpool.tile([P, HPG, D], bf16, tag="wo", name="wo_sb")
            # first quad as two pairs so Qproj(0)'s dt=0 matmul starts sooner
            for d0, d1 in [(0, 2), (2, 4)] + [(NQ * qd, NQ * (qd + 1))
                                              for qd in range(1, DT // NQ)]:
                nc.scalar.dma_start(wq_sb[:, d0:d1, :], wqP[:, d0:d1, :])
            nc.scalar.dma_start(wk_sb[:], wkP[:])
            nc.scalar.dma_start(wv_sb[:], wvP[:])
            nc.scalar.dma_start(wo_sb[:], woP[:])

            ident = consts.tile([P, P], bf16, name="ident")
            masks = consts.tile([P, J4, 512], bf16, name="masks")
            make_identity(nc, ident)
            # causal masks for the diagonal 128x512 tiles: keep (k <= q)
            # i.e. mask_r[i, jq] = 1 iff jq - i - 128 r >= 0
            nc.gpsimd.memset(masks[:], 1.0)
            for r in range(J4):
                nc.gpsimd.affine_select(
                    out=masks[:, r, :], in_=masks[:, r, :],
                    compare_op=mybir.AluOpType.is_ge,
                    fill=0.0, base=-P * r, channel_multiplier=-1,
                    pattern=[[1, 512]])

            xkT = xkv_pool.tile([P, S], bf16, tag="xkT", name="xkT")    # [c,k]
            xv = xkv_pool.tile([P, ST, C], bf16, tag="xv", name="xv")   # [k%128,kc,c]

            at_prev = None
            for j in range(J4):
                sl_ = slice(j * 512, (j + 1) * 512)
                xqT_j = xq_pool.tile([P, HPG, 512], bf16, tag="xqT", name="xqT_j")

                def qproj():
                    # at startup only wq chunks + q quads must land before
                    # PE can work, so Qproj leads at j=0
                    ps_q = [ps8.tile([P, 512], f32, tag="x", name=f"psq{h_}")
                            for h_ in range(HPG)]
                    chunks = ([(0, 2), (2, 4)] if j == 0 else [(0, NQ)]) + \
                        [(NQ * qd, NQ * (qd + 1)) for qd in range(1, DT // NQ)]
                    for d0, d1 in chunks:
                        qc = qstream.tile([P, NQ, 512], bf16, tag="qc", name="qc")
                        nc.sync.dma_start(qc[:, :d1 - d0, :], qT_r[:, d0:d1, sl_])
                        for i in range(d1 - d0):
                            dt = d0 + i
                            for h in range(HPG):
                                nc.tensor.matmul(
                                    ps_q[h][:], wq_sb[:, dt, h * P:(h + 1) * P],
                                    qc[:, i, :], start=(dt == 0),
                                    stop=(dt == DT - 1))
                    for h in range(HPG):
                        nc.vector.tensor_copy(xqT_j[:, h, :], ps_q[h][:])

                def kvproj():
                    # KV leads for j>=1: stream buffers recycle at tile
                    # start, so the next tile's transfers spread into the
                    # B-phase DMA lull instead of bunching just-in-time
                    ps_k = ps_kv_pool.tile([P, 512], f32, tag="kv", name="ps_k")
                    ps_v = ps_kv_pool.tile([P, 512], f32, tag="kv", name="ps_v")
                    for qd in range(DT // NQ):
                        kc = kvstream.tile([P, NQ, 512], bf16, tag="kc", name="kc")
                        vc = kvstream.tile([P, NQ, 512], bf16, tag="vc", name="vc")
                        nc.sync.dma_start(kc[:], kT_r[:, NQ * qd:NQ * (qd + 1), sl_])
                        nc.sync.dma_start(vc[:], vT_r[:, NQ * qd:NQ * (qd + 1), sl_])
                        for i in range(NQ):
                            dt = NQ * qd + i
                            st, sp = dt == 0, dt == DT - 1
                            nc.tensor.matmul(ps_k[:], wk_sb[:, dt, :], kc[:, i, :],
                                             start=st, stop=sp)
                            nc.tensor.matmul(ps_v[:], wv_sb[:, dt, :], vc[:, i, :],
                                             start=st, stop=sp)
                    nc.vector.tensor_copy(xkT[:, sl_], ps_k[:])
                    xvT_sb = vtr.tile([P, 512], bf16, tag="xvT", name="xvT_sb")
                    nc.vector.tensor_copy(xvT_sb[:], ps_v[:])
                    for sc in range(4):
                        pst = ps_kv_pool.tile([P, P], bf16, tag="kv", name="pst")
                        nc.tensor.transpose(pst[:], xvT_sb[:, sc * P:(sc + 1) * P],
                                            ident[:])
                        nc.vector.tensor_copy(xv[:, j * 4 + sc, :], pst[:])

                if j == 0:
                    qproj(); kvproj()
                else:
                    kvproj(); qproj()

                if not do_B:
                    continue
                nk = 4 * (j + 1)
                at_block = apool.tile([P, HPG, 512], bf16, tag="attnT",
                                      name="at_block")

                def score_prob(kc_i, h, sc_pool=None, sc_tag="sc"):
                    # diagonal tiles (r >= 0): columns < 128 r are fully
                    # masked -> compute only cols >= 128 r
                    r = kc_i - 4 * j
                    c0 = 128 * r if r > 0 else 0
                    pool_ = sc_pool or ps_sc_pool
                    ps_s = pool_.tile([P, 512], f32, tag=sc_tag, name="ps_s")
                    nc.tensor.matmul(
                        ps_s[:, c0:], xkT[:, kc_i * P:(kc_i + 1) * P],
                        xqT_j[:, h, c0:], start=True, stop=True)
                    pr = probs.tile([P, 512], bf16, tag="pr", name="pr")
                    nc.scalar.activation(
                        pr[:, c0:], ps_s[:, c0:],
                        mybir.ActivationFunctionType.Sigmoid,
                        scale=float(SCALE))
                    if r >= 0:
                        nc.vector.tensor_mul(
                            out=pr[:, c0:], in0=pr[:, c0:], in1=masks[:, r, c0:])
                    return pr, c0

                # kc-outer: the 4 heads' score matmuls share the xkT[kc]
                # stationary and their AVs share xv[kc] (fewer distinct
                # weight loads); 4 sigmoids of slack decouple ACT latency.
                # C(j-1) groups are spread across the kc loop; they use only
                # the kv PSUM pair since ps8 holds the 4 attn accumulators.
                if _OPTS.get("b_order", "kc") == "kc" or at_prev is None:
                    ps_at = [ps8.tile([P, 512], f32, tag="x", name=f"ps_at{h_}")
                             for h_ in range(HPG)]
                    cpoints = {}
                    if do_C and _OPTS["c_interleave"] and at_prev is not None:
                        cpoints = {(nk * (i + 1)) // 4 - 1: i for i in range(4)}
                    for kc_i in range(nk):
                        if at_prev is None:
                            # B(0): the kv PSUM pair is idle, double the
                            # score pipeline with it
                            prs = [score_prob(kc_i, h,
                                              sc_pool=(ps_kv_pool if h % 2 else None),
                                              sc_tag=("kv" if h % 2 else "sc"))
                                   for h in range(HPG)]
                        else:
                            prs = [score_prob(kc_i, h) for h in range(HPG)]
                        for h in range(HPG):
                            pr, c0 = prs[h]
                            nc.tensor.matmul(ps_at[h][:, c0:], xv[:, kc_i, :],
                                             pr[:, c0:], start=(kc_i == 0),
                                             stop=(kc_i == nk - 1))
                        if kc_i in cpoints:
                            emit_c(at_prev, j - 1, cpoints[kc_i], kv_only=True)
                    for h in range(HPG):
                        nc.vector.tensor_copy(at_block[:, h, :], ps_at[h][:])
                else:
                    # h-outer: one attn accumulator, C(j-1) per-head
                    for h in range(HPG):
                        ps_at = ps8.tile([P, 512], f32, tag="x", name="ps_at")
                        for kc_i in range(nk):
                            pr, c0 = score_prob(kc_i, h)
                            nc.tensor.matmul(ps_at[:, c0:], xv[:, kc_i, :],
                                             pr[:, c0:], start=(kc_i == 0),
                                             stop=(kc_i == nk - 1))
                        nc.vector.tensor_copy(at_block[:, h, :], ps_at[:])
                        if do_C and _OPTS["c_interleave"] and at_prev is not None:
                            emit_c(at_prev, j - 1, h)
                if do_C and not _OPTS["c_interleave"]:
                    for s16 in range(4):
                        emit_c(at_block, j, s16)
                at_prev = at_block

            if do_B and do_C and _OPTS["c_interleave"]:
                for s16 in range(4):
                    emit_c(at_prev, J4 - 1, s16, last=True)

          if internal_io:
              dt_ = consts.tile([1, 1], f32, name="dt_")
              nc.sync.dma_start(dt_[:], dummy_in[:])
              nc.sync.dma_start(dummy_out[:], dt_[:])
          if n_iters:
              import os as _os
              _kw = {}
              if _os.environ.get("LOOP_HINTS") == "1":
                  _kw = dict(hint_engines=(mybir.EngineType.PE,
                                           mybir.EngineType.Activation,
                                           mybir.EngineType.DVE,
                                           mybir.EngineType.Pool,
                                           mybir.EngineType.SP))
              if _os.environ.get("LOOP_STAGGER") == "1":
                  _kw["staggered_reset"] = True
              with tc.For_i(0, n_iters, 1, **_kw):
                  body()
          else:
              body()
    nc.compile()
    return nc


def _get_module():
    if "nc" not in _CACHE:
        _CACHE["nc"] = _build_module()
    return _CACHE["nc"]


def _bf16(a: np.ndarray) -> np.ndarray:
    return np.ascontiguousarray(a.astype(ml_dtypes.bfloat16))


def _pack_w(wT: np.ndarray, free: int) -> np.ndarray:
    """[D, free] weight (already W.T slice) -> SBUF-layout [128, DT, free]."""
    return _bf16(wT.reshape(DT, P, free).transpose(1, 0, 2))


def make_in_maps(query, key, value, Wq, Wk, Wv, Wo):
    """Host-side sharding: per-core input dict (core = b*4 + g)."""
    query = np.asarray(query, dtype=np.float32)
    key = np.asarray(key, dtype=np.float32)
    value = np.asarray(value, dtype=np.float32)
    Wq = np.asarray(Wq, dtype=np.float32)
    Wk = np.asarray(Wk, dtype=np.float32)
    Wv = np.asarray(Wv, dtype=np.float32)
    Wo = np.asarray(Wo, dtype=np.float32)

    qT = [_bf16(query[b].T) for b in range(B)]
    kTb = [_bf16(key[b].T) for b in range(B)]
    vTb = [_bf16(value[b].T) for b in range(B)]
    WqT = Wq.T  # [D, NH*C]
    WkT = Wk.T  # [D, NKV*C]
    WvT = Wv.T
    WoT = Wo.T  # [D_in, D_out]

    in_maps = []
    for core in range(8):
        b, g = divmod(core, 4)
        woT_g = WoT[g * F:(g + 1) * F, :]  # [F, D]
        in_maps.append({
            "qT": qT[b],
            "kT": kTb[b],
            "vT": vTb[b],
            "wqP": _pack_w(WqT[:, g * F:(g + 1) * F], F),
            "wkP": _pack_w(WkT[:, g * C:(g + 1) * C], C),
            "wvP": _pack_w(WvT[:, g * C:(g + 1) * C], C),
            # [F, D] -> [128, HPG, D] (partition = c within head chunk)
            "woP": _bf16(woT_g.reshape(HPG, P, D).transpose(1, 0, 2)),
        })
    return in_maps


def kernel(**inputs) -> np.ndarray:
    nc = _get_module()
    in_maps = make_in_maps(**inputs)
    res = run_bass_kernel_spmd(nc, in_maps, core_ids=list(range(8)))
    parts = [np.asarray(res.results[c]["out"], dtype=np.float32)
             for c in range(8)]
    full = np.empty((B, S, D), dtype=np.float32)
    for b in range(B):
        full[b] = parts[b * 4] + parts[b * 4 + 1] + parts[b * 4 + 2] + parts[b * 4 + 3]
    return full
